# revision 25
# baseline (speedup 1.0000x reference)
"""Trainium2 Bass kernel for nn_Attention_54855322304634 (8 NeuronCores).

Strategy (batch x head sharding, no K/V collective):
- core c handles batch b = c//4 and head group g = c%4 (4 of 16 heads),
  over the FULL sequence (2048 rows). Attention is entirely local.
- LayerNorm + AdaLN modulation are FOLDED INTO the QKV matmul:
  x arrives host-pre-transposed as xT [H, S] (bf16). Per-row stats come
  from tiny PE matmuls (lhsT = xT chunk, rhs = ones -> column sums of x
  and x^2 in [row, rt] layout). The per-H-dim factor s1c = ln_w*(scale+1)
  is multiplied into the weights; the per-row mean/rstd corrections enter
  the matmul as 2 augmented contraction rows: xaug = [-mean; 1/rstd],
  waug = [c1; c2] with c1 = sum_d s1c_d*w[d,n], c2 = sum_d shift_d*w[d,n].
  Then qkv = rstd*(psRaw); the rstd scale cancels inside qk-LayerNorm for
  q/k and is applied to v as a per-row tensor_scalar.
- Modulation (all 3072 dims) is computed fully locally on each core (no
  AllGather): modw streams in bf16, 2-col matvec matmuls.
- QKV weights carry 8 extra per-head-sum columns so qk-LN needs only a
  sum-of-squares reduction. RoPE with qn/kn folded into per-row rotation
  factors; 1/sqrt(hd) folded into the exp's scale.
- k/q head transposes go through DRAM and the DMA xbar engine (no PE or
  DVE cost).
- Attention per head: scores_T = K_hT.T @ q_hT, exp with no max
  subtraction, PV with ones-augmented V so the softmax denominator falls
  out of the same accumulation.
- o-proj partials in f32r with gate pre-folded into w_o^T; partial rows
  go to DRAM in bf16 and a per-512-row-block ReduceScatter (x4, pipelined
  against compute) reduces into a scratch that is copied to the output
  tensor (collectives cannot write IO tensors directly).
"""

import sys

if "/opt/trn_rl_repo" not in sys.path:
    sys.path.insert(0, "/opt/trn_rl_repo")

import numpy as np

import concourse.bass as bass
import concourse.tile as tile
from concourse import bacc, mybir
from concourse.bass_utils import run_bass_kernel_spmd

F32 = mybir.dt.float32
F32R = mybir.dt.float32r
BF16 = mybir.dt.bfloat16
AX = mybir.AxisListType
OP = mybir.AluOpType
ACT = mybir.ActivationFunctionType

NH, HD, H, B, S, A = 16, 64, 1024, 2, 2048, 1024
EPS = 1e-5
HPC = 4                  # heads per core
RT = S // 128            # 16 row tiles
RG = 4                   # row groups of 512
KC = S // 128            # 16 key chunks
QC = 4                   # query blocks of 512
# wqkvT columns: [k(256) | ksum(4) | v(256) | q(256) | qsum(4)]
W3 = 776
CK0, CK1 = 0, 260        # k + ksum
CV0, CV1 = 260, 516      # v
CQ0, CQ1 = 516, 776      # q + qsum
GROUPS = [[0, 1, 2, 3], [4, 5, 6, 7]]


def _bc(ap, p):
    """Stride-0 partition broadcast to [p, ...] (DRAM source)."""
    dims = list(ap.ap)
    if dims[0][1] == 1:
        dims = dims[1:]
    return bass.AP(tensor=ap.tensor, offset=ap.offset, ap=[[0, p]] + dims)


def _emit(tc, ins, out, upto="D"):
    nc = tc.nc
    (xT_in, freqs_in, wqkvT_in, woT_in, modwss_in, modwg_in, modb_in,
     ada_in, lnw_in, qnw_in, knw_in) = (
        ins["xT"], ins["freqs"], ins["wqkvT"], ins["woT"], ins["modwss"],
        ins["modwg"], ins["modb"], ins["ada"], ins["lnw"], ins["qnw"],
        ins["knw"],
    )

    const = tc.alloc_tile_pool(name="const", bufs=1)
    pers = tc.alloc_tile_pool(name="pers", bufs=1)
    dram = tc.alloc_tile_pool(name="dram", bufs=1, space="DRAM")

    # ---------------- constants ----------------
    eps128 = const.tile([128, 1], F32)
    nc.vector.memset(eps128, EPS)
    eps64 = const.tile([128, 1], F32)
    nc.vector.memset(eps64, EPS * HD)
    onesb = const.tile([128, 1], BF16)
    nc.vector.memset(onesb, 1.0)
    # warm the Silu activation-table set off the mod critical path
    junk = const.tile([1, 2], F32)
    nc.vector.memset(junk, 0.0)
    nc.scalar.activation(out=junk, in_=junk, func=ACT.Silu)

    # ---------------- bulk loads ----------------
    # SP queue: xT halves, wqkv, freqs, woT (in that order).
    xT = pers.tile([128, 8, S], BF16)
    x_src = xT_in.rearrange("(kt p) s -> p kt s", p=128)
    nc.sync.dma_start(out=xT[:, 0:4, :], in_=x_src[:, 0:4, :])
    nc.sync.dma_start(out=xT[:, 4:8, :], in_=x_src[:, 4:8, :])
    wq = pers.tile([128, 8, W3], BF16)
    nc.sync.dma_start(out=wq, in_=wqkvT_in.rearrange("(kt p) n -> p kt n", p=128))
    f0a = const.tile([128, RT, 32], F32)
    f1a = const.tile([128, RT, 32], F32)
    nc.sync.dma_start(
        out=f0a, in_=freqs_in.rearrange("(rt p) two d -> p rt two d", p=128)[:, :, 0, :])
    nc.sync.dma_start(
        out=f1a, in_=freqs_in.rearrange("(rt p) two d -> p rt two d", p=128)[:, :, 1, :])
    qn_rep = const.tile([128, HD], F32)
    nc.sync.dma_start(out=qn_rep, in_=_bc(qnw_in, 128))
    kn_rep = const.tile([128, HD], F32)
    nc.sync.dma_start(out=kn_rep, in_=_bc(knw_in, 128))
    woT_sb = pers.tile([128, 2, H], F32R)
    nc.sync.dma_start(out=woT_sb, in_=woT_in.rearrange("(hp p) n -> p hp n", p=128).bitcast(F32R))

    # ACT queue: small mod inputs first, then modw chunks.
    lnw_cols = const.tile([128, 8], F32)
    nc.scalar.dma_start(out=lnw_cols, in_=lnw_in[0, :].rearrange("(kt p) -> p kt", p=128))
    ada_sb = const.tile([128, 8], F32)
    nc.scalar.dma_start(out=ada_sb, in_=ada_in)
    modb_sb = const.tile([128, 24], F32)
    nc.scalar.dma_start(out=modb_sb, in_=modb_in)

    # ---------------- modulation (fully local, no collective) -------------
    mod_sb = pers.tile([128, 16], F32)    # scale cols 0-7, shift cols 8-15
    s1c = pers.tile([128, 8], F32)
    shT = mod_sb[:, 8:16]
    gmod = pers.tile([128, 8], F32)       # gate
    silu_sb = pers.tile([128, 8, 2], BF16)
    mw_src = modwss_in.rearrange("(kt p) m -> p kt m", p=128)
    mg_src = modwg_in.rearrange("(kt p) m -> p kt m", p=128)

    gpsum = tc.alloc_tile_pool(name="gpsum", bufs=1, space="PSUM")
    modg_ps = gpsum.tile([128, 8, 2], F32)
    modp = tc.alloc_tile_pool(name="modp", bufs=2)
    modpsum = tc.alloc_tile_pool(name="modpsum", bufs=1, space="PSUM")
    mod_ps = modpsum.tile([128, 16, 2], F32)

    if upto == "L":
        sqp_dummy = None
        modpsum.release(); modp.release(); gpsum.release()
        dram.release(); pers.release(); const.release()
        return

    nc.vector.memset(silu_sb, 0.0)
    nc.scalar.activation(out=silu_sb[:, :, 0], in_=ada_sb, func=ACT.Silu)
    # NOTE: PSUM accumulation chains must be sequential within a bank —
    # interleaved chains accumulate incorrectly.  Hence t-outer, kt-inner
    # (all modw chunks resident).
    mws = []
    for kt in range(8):
        mw = modp.tile([128, 2048], BF16, tag="mw", bufs=8, name=f"mw{kt}")
        nc.scalar.dma_start(out=mw, in_=mw_src[:, kt, :])
        mws.append(mw)
    for t in range(16):
        for kt in range(8):
            nc.tensor.matmul(
                mod_ps[:, t, :], mws[kt][:, t * 128:(t + 1) * 128],
                silu_sb[:, kt, :], start=(kt == 0), stop=(kt == 7))
    # preload the Sqrt table set while ACT is idle (used for stats + qk-LN)
    nc.scalar.activation(out=junk, in_=junk, func=ACT.Sqrt)
    nc.vector.tensor_tensor(
        out=mod_sb, in0=mod_ps[:, :, 0], in1=modb_sb[:, 0:16], op=OP.add)
    nc.vector.tensor_scalar_add(s1c, mod_sb[:, 0:8], 1.0)
    nc.vector.tensor_tensor(out=s1c, in0=s1c, in1=lnw_cols, op=OP.mult)

    if upto == "mod0":
        modpsum.release(); modp.release(); gpsum.release()
        dram.release(); pers.release(); const.release()
        return

    # gate chunk loads (lower priority; needed only by o-proj fold)
    mg = modp.tile([128, 8, 1024], BF16, tag="mg", bufs=1)
    nc.scalar.dma_start(out=mg[:, 0:4, :], in_=mg_src[:, 0:4, :])
    nc.scalar.dma_start(out=mg[:, 4:8, :], in_=mg_src[:, 4:8, :])

    # ---------------- x row-stats via tiny PE matmuls ----------------
    # rt-outer so each accumulation chain is sequential within the bank.
    psS = modpsum.tile([128, 32], F32)    # [:,0:16] sum x, [:,16:32] sum x^2
    sqp = tc.alloc_tile_pool(name="sqp", bufs=1)
    sq = sqp.tile([128, 8, S], BF16)
    for kt in range(8):
        nc.vector.tensor_tensor(out=sq[:, kt, :], in0=xT[:, kt, :],
                                in1=xT[:, kt, :], op=OP.mult)
    for rt in range(RT):
        for kt in range(8):
            nc.tensor.matmul(psS[:, rt:rt + 1],
                             xT[:, kt, rt * 128:(rt + 1) * 128], onesb,
                             start=(kt == 0), stop=(kt == 7))
    for rt in range(RT):
        for kt in range(8):
            nc.tensor.matmul(psS[:, 16 + rt:17 + rt],
                             sq[:, kt, rt * 128:(rt + 1) * 128], onesb,
                             start=(kt == 0), stop=(kt == 7))

    if upto == "stats":
        sqp.release(); modpsum.release(); modp.release(); gpsum.release()
        dram.release(); pers.release(); const.release()
        return

    # ---------------- c1/c2 + weight fold ----------------
    s1c_bf = pers.tile([128, 8], BF16)
    nc.vector.tensor_scalar_add(s1c_bf, s1c, 0.0)
    shT_bf = pers.tile([128, 8], BF16)
    nc.vector.tensor_scalar_add(shT_bf, shT, 0.0)
    ccps1 = modpsum.tile([1, 2, 512], F32)
    ccps2 = modpsum.tile([1, 2, 512], F32)
    cuts = [(0, 512), (512, W3)]
    for ccps, lhs in ((ccps1, s1c_bf), (ccps2, shT_bf)):
        for ci, (c0, c1) in enumerate(cuts):
            for kt in range(8):
                nc.tensor.matmul(ccps[:, ci, 0:c1 - c0], lhs[:, kt:kt + 1],
                                 wq[:, kt, c0:c1],
                                 start=(kt == 0), stop=(kt == 7))
    wstage = pers.tile([1, 2 * W3], BF16)
    for i, ccps in enumerate((ccps1, ccps2)):
        for ci, (c0, c1) in enumerate(cuts):
            nc.scalar.copy(out=wstage[:, i * W3 + c0:i * W3 + c1],
                           in_=ccps[:, ci, 0:c1 - c0])
    waug_d = dram.tile([2, W3], BF16)
    nc.scalar.dma_start(out=waug_d[:].rearrange("a b -> (a b)")[None, :], in_=wstage)
    waug = pers.tile([2, W3], BF16)
    nc.scalar.dma_start(out=waug, in_=waug_d)
    # fold s1c into the weights (per contraction-dim scale)
    for kt in range(8):
        eng = nc.vector if kt % 2 == 0 else nc.gpsimd
        eng.tensor_scalar(out=wq[:, kt, :], in0=wq[:, kt, :],
                          scalar1=s1c[:, kt:kt + 1], scalar2=None, op0=OP.mult)

    if upto == "cc":
        sqp.release(); modpsum.release(); modp.release(); gpsum.release()
        dram.release(); pers.release(); const.release()
        return

    negm_f = pers.tile([128, RT], F32)
    nc.vector.tensor_scalar_mul(negm_f, psS[:, 0:16], -1.0 / H)
    ex2 = pers.tile([128, RT], F32)
    nc.vector.tensor_scalar_mul(ex2, psS[:, 16:32], 1.0 / H)
    msq = pers.tile([128, RT], F32)
    nc.scalar.activation(out=msq, in_=negm_f, func=ACT.Square)
    var_x = pers.tile([128, RT], F32)
    nc.vector.tensor_tensor(out=var_x, in0=ex2, in1=msq, op=OP.subtract)
    irstd_f = pers.tile([128, RT], F32)
    nc.scalar.activation(out=irstd_f, in_=var_x, func=ACT.Sqrt, bias=eps128)
    rstd_x = pers.tile([128, RT], F32)
    nc.vector.reciprocal(rstd_x, irstd_f)
    negm_bf = pers.tile([128, RT], BF16)
    nc.gpsimd.tensor_scalar_add(negm_bf, negm_f, 0.0)
    irstd_bf = pers.tile([128, RT], BF16)
    nc.gpsimd.tensor_scalar_add(irstd_bf, irstd_f, 0.0)

    # roundtrip [-m; 1/rstd] through DRAM into augmented-row layout
    stat_d = dram.tile([2, S], BF16)
    nc.scalar.dma_start(
        out=stat_d[0, :].rearrange("(t p) -> p t", p=128), in_=negm_bf)
    nc.scalar.dma_start(
        out=stat_d[1, :].rearrange("(t p) -> p t", p=128), in_=irstd_bf)
    xaug = pers.tile([2, S], BF16)
    nc.scalar.dma_start(out=xaug, in_=stat_d)

    if upto == "mod":
        sqp.release(); modpsum.release(); modp.release(); gpsum.release()
        dram.release(); pers.release(); const.release()
        return

    sqp.release()
    modpsum.release()

    # ---------------- rope factors (qn/kn folded) ----------------
    gfac = {}
    for is_q in (True, False):
        w_rep = qn_rep if is_q else kn_rep
        we, wo_ = w_rep[:, 0::2], w_rep[:, 1::2]
        g = [const.tile([128, RT, 32], F32, name=f"g{is_q}{i}") for i in range(4)]
        for i, (fa, wv) in enumerate(((f0a, we), (f1a, wo_), (f0a, wo_), (f1a, we))):
            nc.vector.tensor_tensor(
                out=g[i], in0=fa,
                in1=wv[:, None, :].to_broadcast((128, RT, 32)), op=OP.mult)
        gfac[is_q] = g

    # ---------------- phase B: k-pass then q-pass ----------------
    kT = pers.tile([128, 2, S], BF16)        # [2-head*64, pair, rows]
    qT = pers.tile([128, 2, S], BF16)
    vsb = pers.tile([128, KC, HPC * 65], BF16)
    nc.vector.memset(vsb, 1.0)               # ones col 64 of each head block
    oT = pers.tile([128, 2, S], F32R)        # [2-head*64, pair, rows]
    hk = dram.tile([S, 256], BF16)
    hq = dram.tile([S, 256], BF16)

    with tc.tile_pool(name="bps", bufs=3, space="PSUM") as bps, \
         tc.tile_pool(name="bwork", bufs=4) as work, \
         tc.tile_pool(name="bstats", bufs=6) as stats2:

        def qkv_pass(is_q):
            hdst = hq if is_q else hk
            dstT = qT if is_q else kT
            c0a, c1a = (CQ0, CQ1) if is_q else (CK0, CK1)
            for pr_i in range(RT // 2):
                rts = (2 * pr_i, 2 * pr_i + 1)
                ps1s, ps2s = [], []
                for rt in rts:
                    ps1 = bps.tile([128, 260], F32, tag="ps1", name=f"ps1_{is_q}_{rt}")
                    for kt in range(8):
                        nc.tensor.matmul(
                            ps1, xT[:, kt, rt * 128:(rt + 1) * 128],
                            wq[:, kt, c0a:c1a], start=(kt == 0), stop=False)
                    nc.tensor.matmul(
                        ps1, xaug[:, rt * 128:(rt + 1) * 128],
                        waug[:, c0a:c1a], start=False, stop=True)
                    ps1s.append(ps1)
                    if not is_q:
                        ps2 = bps.tile([128, 256], F32, tag="ps2", name=f"ps2_{rt}")
                        for kt in range(8):
                            nc.tensor.matmul(
                                ps2, xT[:, kt, rt * 128:(rt + 1) * 128],
                                wq[:, kt, CV0:CV1], start=(kt == 0), stop=False)
                        nc.tensor.matmul(
                            ps2, xaug[:, rt * 128:(rt + 1) * 128],
                            waug[:, CV0:CV1], start=False, stop=True)
                        ps2s.append(ps2)
                # interleave the gate matmuls after the first k-pair so the
                # PE queue never stalls on the modw_g load
                if not is_q and pr_i == 1:
                    emit_gate()

                # batched qk-LN stats for the pair ([128, 8] = 2 tiles x 4 heads)
                negmean = stats2.tile([128, 8], F32, tag="negmean")
                for i in range(2):
                    nc.vector.tensor_scalar_mul(
                        negmean[:, 4 * i:4 * i + 4], ps1s[i][:, 256:260], -1.0 / HD)
                if not is_q:
                    for i, rt in enumerate(rts):
                        vdst = vsb[:, rt, :].rearrange("p (h c) -> p h c", c=65)[:, :, 0:64]
                        vsrc = ps2s[i].rearrange("p (h d) -> p h d", h=4)
                        if i == 0:
                            nc.scalar.activation(
                                out=vdst, in_=vsrc, func=ACT.Identity,
                                scale=rstd_x[:, rt:rt + 1])
                        else:
                            nc.vector.tensor_scalar(
                                out=vdst, in0=vsrc,
                                scalar1=rstd_x[:, rt:rt + 1], scalar2=None,
                                op0=OP.mult)
                sq = work.tile([128, 2, 256], F32, tag="sq2")
                for i in range(2):
                    nc.scalar.activation(out=sq[:, i, :], in_=ps1s[i][:, 0:256],
                                         func=ACT.Square)
                s2 = stats2.tile([128, 8], F32, tag="s2")
                nc.vector.tensor_reduce(
                    out=s2, in_=sq[:].rearrange("p a (h d) -> p (a h) d", d=64),
                    axis=AX.X, op=OP.add)
                m64 = stats2.tile([128, 8], F32, tag="m64")
                nc.scalar.activation(out=m64, in_=negmean, func=ACT.Square, scale=8.0)
                var = stats2.tile([128, 8], F32, tag="var")
                nc.gpsimd.tensor_tensor(out=var, in0=s2, in1=m64, op=OP.subtract)
                rstd = stats2.tile([128, 8], F32, tag="rstd8")
                nc.scalar.activation(out=rstd, in_=var, func=ACT.Sqrt, bias=eps64)
                nc.vector.reciprocal(rstd, rstd)
                nbias = stats2.tile([128, 8], F32, tag="nbias")
                nc.vector.tensor_tensor(out=nbias, in0=negmean, in1=rstd, op=OP.mult)

                # normalize 4 head-slices per tile: 2 on ACT, 2 on DVE
                ys = [work.tile([128, 256], F32, tag="y", name=f"y{is_q}_{rts[i]}")
                      for i in range(2)]
                for hh in range(4):
                    for i in range(2):
                        sl = slice(hh * 64, (hh + 1) * 64)
                        c = 4 * i + hh
                        if hh < 2:
                            nc.scalar.activation(
                                out=ys[i][:, sl], in_=ps1s[i][:, sl], func=ACT.Identity,
                                scale=rstd[:, c:c + 1], bias=nbias[:, c:c + 1])
                        else:
                            nc.vector.tensor_scalar(
                                out=ys[i][:, sl], in0=ps1s[i][:, sl],
                                scalar1=negmean[:, c:c + 1], scalar2=rstd[:, c:c + 1],
                                op0=OP.add, op1=OP.mult)

                # rope: re-chain on DVE, im-chain on gpsimd
                g = gfac[is_q]
                ros = []
                for i, rt in enumerate(rts):
                    y = ys[i]
                    ro = work.tile([128, 256], BF16, tag="ro", name=f"ro{is_q}_{rt}")
                    tm = work.tile([128, 256], F32, tag="tm", name=f"tm{is_q}_{rt}")
                    gb = [gi[:, rt, :][:, None, :].to_broadcast((128, 4, 32))
                          for gi in g]
                    y4 = y[:].rearrange("p (h d2 two) -> p h d2 two", h=4, two=2)
                    ro4 = ro[:].rearrange("p (h d2 two) -> p h d2 two", h=4, two=2)
                    tm_re = tm[:, 0:128].rearrange("p (h d2) -> p h d2", h=4)
                    tm_im = tm[:, 128:256].rearrange("p (h d2) -> p h d2", h=4)
                    e_re = nc.vector if i == 0 else nc.gpsimd
                    e_im = nc.gpsimd if i == 0 else nc.vector
                    e_re.tensor_tensor(out=ro4[:, :, :, 0], in0=y4[:, :, :, 0], in1=gb[0], op=OP.mult)
                    e_re.tensor_tensor(out=tm_re, in0=y4[:, :, :, 1], in1=gb[1], op=OP.mult)
                    e_re.tensor_tensor(out=ro4[:, :, :, 0], in0=ro4[:, :, :, 0], in1=tm_re, op=OP.subtract)
                    e_im.tensor_tensor(out=ro4[:, :, :, 1], in0=y4[:, :, :, 1], in1=gb[2], op=OP.mult)
                    e_im.tensor_tensor(out=tm_im, in0=y4[:, :, :, 0], in1=gb[3], op=OP.mult)
                    e_im.tensor_tensor(out=ro4[:, :, :, 1], in0=ro4[:, :, :, 1], in1=tm_im, op=OP.add)
                    ros.append(ro)
                    nc.sync.dma_start(out=hdst[rt * 128:(rt + 1) * 128, :], in_=ro)
                if pr_i % 2 == 1:
                    rg = pr_i // 2
                    for pair in range(2):
                        nc.sync.dma_start_transpose(
                            out=dstT[:, pair, rg * 512:(rg + 1) * 512],
                            in_=hdst[rg * 512:(rg + 1) * 512, pair * 128:(pair + 1) * 128])

        def emit_gate():
            for t in range(8):
                for gk in range(8):      # gate contraction chunk (128 dims)
                    nc.tensor.matmul(
                        modg_ps[:, t, :],
                        mg[:, gk, t * 128:(t + 1) * 128],
                        silu_sb[:, gk, :], start=(gk == 0), stop=(gk == 7))

        qkv_pass(False)
        # gate -> woT fold (only needed by o-proj, well into phase C)
        gate_d = dram.tile([1, H], F32)
        nc.vector.tensor_tensor(out=gmod, in0=modg_ps[:, :, 0],
                                in1=modb_sb[:, 16:24], op=OP.add)
        nc.scalar.dma_start(
            out=gate_d[0, :].rearrange("(t p) -> p t", p=128), in_=gmod)
        g_rep = pers.tile([128, H], F32)
        nc.scalar.dma_start(out=g_rep, in_=_bc(gate_d[:], 128))
        wo_f = woT_sb[:].bitcast(F32)
        nc.gpsimd.tensor_tensor(
            out=woT_sb[:], in0=wo_f,
            in1=g_rep[:, None, :].to_broadcast((128, 2, H)), op=OP.mult)
        qkv_pass(True)
        # warm the Exp table set before phase C
        nc.scalar.activation(out=junk, in_=junk, func=ACT.Exp)

    if upto == "dbg1":
        nc.sync.dma_start(out=out[0:128, :].bitcast(BF16)[:, 0:1024],
                          in_=kT[:, 0, 0:1024])
        nc.sync.dma_start(out=out[128:256, :].bitcast(BF16)[:, 0:1024],
                          in_=qT[:, 0, 0:1024])
        nc.sync.dma_start(out=out[256:384, :].bitcast(BF16)[:, 0:1024],
                          in_=vsb[:, 0:8, 0:128])
        nc.sync.dma_start(out=out[384:512, 0:16].bitcast(F32)[:, 0:8],
                          in_=s1c)
        nc.sync.dma_start(out=out[384:512, 16:32].bitcast(F32)[:, 0:8],
                          in_=mod_sb[:, 8:16])
        nc.sync.dma_start(out=out[384:512, 32:64].bitcast(F32)[:, 0:16],
                          in_=negm_f)
        nc.sync.dma_start(out=out[384:512, 64:96].bitcast(F32)[:, 0:16],
                          in_=irstd_f)

    modp.release()
    gpsum.release()

    if upto in ("A", "B", "dbg1"):
        dram.release(); pers.release(); const.release()
        return

    # ---------------- phase C: attention + o-proj + ReduceScatter ----------
    psout = dram.tile([S, H], BF16)
    rs_out = dram.tile([QC, 128, H], BF16)
    kcg = [2] * 8                            # exp batching groups over KC

    with tc.tile_pool(name="spsum", bufs=2, space="PSUM") as spsum, \
         tc.tile_pool(name="popool", bufs=3, space="PSUM") as popool, \
         tc.tile_pool(name="pppool", bufs=1, space="PSUM") as pppool, \
         tc.tile_pool(name="esb", bufs=6) as esb, \
         tc.tile_pool(name="recp", bufs=3) as recp:
        def emit_oproj(qc, part=None):
            # o-proj partial + ReduceScatter for a finished 512-row block
            rt2s = range(4) if part is None else ([0, 1] if part == 0 else [2, 3])
            for rt2 in rt2s:
                rs = slice(qc * 512 + rt2 * 128, qc * 512 + rt2 * 128 + 128)
                ppsb = recp.tile([128, H], BF16, tag="ppsb",
                                 name=f"ppsb{qc}_{rt2}")
                for nch in range(2):
                    pp = pppool.tile([128, 512], F32, tag="pp",
                                    name=f"pp{qc}_{rt2}_{nch}")
                    for hp in range(2):
                        nc.tensor.matmul(
                            pp, oT[:, hp, rs],
                            woT_sb[:, hp, nch * 512:(nch + 1) * 512],
                            start=(hp == 0), stop=(hp == 1))
                    nc.vector.tensor_copy(
                        out=ppsb[:, nch * 512:(nch + 1) * 512], in_=pp)
                nc.sync.dma_start(out=psout[rs, :], in_=ppsb)
            if part in (None, 1) and "noag" not in upto:
                nc.gpsimd.collective_compute(
                    "ReduceScatter", OP.add,
                    ins=[psout[qc * 512:(qc + 1) * 512, :].opt()],
                    outs=[rs_out[qc, :, :].opt()],
                    replica_groups=GROUPS,
                )

        for qc in range(QC):
            qs = slice(qc * 512, (qc + 1) * 512)
            for h in range(HPC):
                hp, lo = h // 2, (h % 2) * 64
                q_h = qT[lo:lo + 64, hp, qs]
                po = popool.tile([128, 512], F32, tag="po", name=f"po{qc}_{h}")
                kc = 0
                for gi, gsz in enumerate(kcg):
                    ps = spsum.tile([128, 2, 512], F32, tag="sps",
                                    name=f"sps{qc}_{h}_{gi}")
                    for j in range(gsz):
                        nc.tensor.matmul(
                            ps[:, j, :],
                            kT[lo:lo + 64, hp, (kc + j) * 128:(kc + j + 1) * 128],
                            q_h, start=True, stop=True)
                    et = esb.tile([128, 2, 512], BF16, tag="et",
                                  name=f"et{qc}_{h}_{gi}")
                    # qk-LN folded a 1/8 into each of q and k; 0.125*64 = 8
                    nc.scalar.activation(
                        out=et[:, 0:gsz, :], in_=ps[:, 0:gsz, :], func=ACT.Exp,
                        scale=8.0)
                    for j in range(gsz):
                        vcol = h * 65
                        nc.tensor.matmul(
                            po[0:65, :], vsb[:, kc + j, vcol:vcol + 65],
                            et[:, j, :], start=(kc + j == 0),
                            stop=(kc + j == KC - 1))
                    kc += gsz
                rec = recp.tile([128, 512], F32, tag="rec", name=f"rec{qc}_{h}")
                nc.vector.reciprocal(rec[64:65, :], po[64:65, :])
                dden = dram.tile([1, 512], F32, tag=f"dden{(qc * HPC + h) % 4}",
                                 name=f"dden{qc}_{h}")
                nc.sync.dma_start(out=dden, in_=rec[64:65, :])
                recb = recp.tile([64, 512], F32, tag="recb", name=f"recb{qc}_{h}")
                nc.sync.dma_start(out=recb, in_=_bc(dden[:], 64))
                nc.vector.tensor_tensor(
                    out=oT[lo:lo + 64, hp, qs], in0=po[0:64, :], in1=recb,
                    op=OP.mult)
                if qc > 0 and h == 0:
                    emit_oproj(qc - 1, part=0)   # keeps PE dense at boundary
                if qc > 0 and h == 1:
                    emit_oproj(qc - 1, part=1)
            if qc == QC - 1:
                emit_oproj(qc)

        if "noag" not in upto:
            for qc in range(QC):
                nc.sync.dma_start(out=out[qc * 128:(qc + 1) * 128, :],
                                  in_=rs_out[qc, :, :])

    dram.release()
    pers.release()
    const.release()


_CACHE = {}


def _build(upto="D"):
    if ("nc", upto) in _CACHE:
        return _CACHE[("nc", upto)]
    nc = bacc.Bacc("TRN2", target_bir_lowering=False, debug=False,
                   enable_asserts=False, num_devices=8)
    ins = {
        "xT": nc.dram_tensor("xT", [H, S], BF16, kind="ExternalInput").ap(),
        "freqs": nc.dram_tensor("freqs", [S, 2, 32], F32, kind="ExternalInput").ap(),
        "wqkvT": nc.dram_tensor("wqkvT", [H, W3], BF16, kind="ExternalInput").ap(),
        "woT": nc.dram_tensor("woT", [2 * 128, H], F32, kind="ExternalInput").ap(),
        "modwss": nc.dram_tensor("modwss", [H, 2048], BF16, kind="ExternalInput").ap(),
        "modwg": nc.dram_tensor("modwg", [H, 1024], BF16, kind="ExternalInput").ap(),
        "modb": nc.dram_tensor("modb", [128, 24], F32, kind="ExternalInput").ap(),
        "ada": nc.dram_tensor("ada", [128, 8], F32, kind="ExternalInput").ap(),
        "lnw": nc.dram_tensor("lnw", [1, H], F32, kind="ExternalInput").ap(),
        "qnw": nc.dram_tensor("qnw", [1, HD], F32, kind="ExternalInput").ap(),
        "knw": nc.dram_tensor("knw", [1, HD], F32, kind="ExternalInput").ap(),
    }
    out = nc.dram_tensor("out", [512, H], BF16, kind="ExternalOutput").ap()
    with tile.TileContext(nc) as tc:
        _emit(tc, ins, out, upto=upto)
    nc.compile()
    _CACHE[("nc", upto)] = nc
    return nc


def _shard(inputs):
    import ml_dtypes
    bf16 = ml_dtypes.bfloat16
    x = np.asarray(inputs["x"], np.float32).reshape(B, S, H)
    ada = np.asarray(inputs["ada_cond"], np.float32)
    freqs = np.ascontiguousarray(
        np.asarray(inputs["freqs"], np.float32).transpose(0, 2, 1))  # [S,2,32]
    wqkv = np.asarray(inputs["w_qkv"], np.float32)      # [3H, H]
    wo = np.asarray(inputs["w_o"], np.float32)          # [H, H]
    modw = np.asarray(inputs["mod_w"], np.float32)
    modb = np.asarray(inputs["mod_b"], np.float32)
    lnw = np.asarray(inputs["ln_w"], np.float32).reshape(1, H)
    qnw = np.asarray(inputs["qn_w"], np.float32).reshape(1, HD)
    knw = np.asarray(inputs["kn_w"], np.float32).reshape(1, HD)

    modwT = np.ascontiguousarray(modw.T)                 # [1024, 3072]
    modwss = np.ascontiguousarray(modwT[:, 0:2048]).astype(bf16)
    modwg = np.ascontiguousarray(modwT[:, 2048:3072]).astype(bf16)
    modb24 = np.ascontiguousarray(modb.reshape(24, 128).T)  # [128, 24]

    in_maps = []
    for c in range(8):
        b, g = c // 4, c % 4
        hs = slice(g * 256, (g + 1) * 256)
        krows = wqkv[H:2 * H][hs]                        # [256, H]
        qrows = wqkv[0:H][hs]
        vrows = wqkv[2 * H:3 * H][hs]
        ksums = np.stack([krows[i * 64:(i + 1) * 64].sum(0) for i in range(4)])
        qsums = np.stack([qrows[i * 64:(i + 1) * 64].sum(0) for i in range(4)])
        # columns: [k(256) | ksum(4) | v(256) | q(256) | qsum(4)]
        wfull = np.concatenate([krows, ksums, vrows, qrows, qsums], 0)  # [776, H]
        in_maps.append({
            "xT": np.ascontiguousarray(x[b].T).astype(bf16),          # [H, S]
            "freqs": freqs,
            "wqkvT": np.ascontiguousarray(wfull.T).astype(bf16),
            "woT": np.ascontiguousarray(wo.T[hs]),                    # [256, H]
            "modwss": modwss,
            "modwg": modwg,
            "modb": modb24,
            "ada": np.ascontiguousarray(ada[b].reshape(8, 128).T),
            "lnw": lnw, "qnw": qnw, "knw": knw,
        })
    return in_maps


def _unshard(results):
    full = np.empty((B, S, H), np.float32)
    for b in range(B):
        for i in range(4):
            r = np.asarray(results[4 * b + i]["out"], np.float32)  # [512, H]
            for qc in range(QC):
                full[b, qc * 512 + i * 128: qc * 512 + (i + 1) * 128] = \
                    r[qc * 128:(qc + 1) * 128]
    return full


def _run(inputs, **kw):
    nc = _build()
    res = run_bass_kernel_spmd(nc, _shard(inputs), core_ids=list(range(8)), **kw)
    return _unshard(res.results), res


def kernel(**inputs) -> np.ndarray:
    out, _ = _run(inputs)
    return out
